# revision 10
# baseline (speedup 1.0000x reference)
"""Trainium2 Bass kernel for a 3-layer LSTM (B=64, T=256, F=64, H=1024)
+ tanh output projection, SPMD across 8 NeuronCores.

Strategy: shard the 4H=4096 gate dimension 8 ways (each core owns a
512-wide gate slice == a 128-wide h-slice per layer), keep the full batch
B=64 on every core. Per time step each core computes its gate slice
(h.T stationary [128,64] x weight moving [128,512] matmuls, PSUM
accumulated over K), does the LSTM elementwise on its slice, transposes
its h-slice to [128,B] via the PE, and exchanges it so every core has
the full h.T for the next step. The 3 layers are software-pipelined:
slot s computes L1(t=s), L2(t=s-1), L3(t=s-2). All three layers'
transposed h-slices for a slot are packed into ONE [128,192] buffer and
exchanged with a single AllGather per slot (collective dispatch
overhead dominates, so 1 merged AG/slot beats 3 small ones 3x). All
matmul operands are bf16 (fp32 matmul runs at 1/4 rate on the PE);
accumulation, elementwise and state stay fp32. The output projection
runs every 8 slots from SBUF-resident gathered h3.
"""

import sys

sys.path.insert(0, "/opt/trn_rl_repo")

import numpy as np
import ml_dtypes

BF16 = ml_dtypes.bfloat16

B, T, F, H = 64, 256, 64, 1024
R = 8           # cores
G = 4 * H // R  # 512 gate slice per core
HS = H // R     # 128 h slice per core
YG = 8          # y-projection group size (slots)

_GATE_ORDER = (0, 1, 3, 2)  # i, f, o, g (PyTorch row blocks i,f,g,o)


def _slice_rows(w, r):
    """Rows of a [4H, *] weight for core r, in i|f|o|g block order."""
    return np.concatenate([w[g * H + HS * r: g * H + HS * (r + 1)] for g in _GATE_ORDER], 0)


def _k_tiles(wT):
    """[K, G] -> [128, K/128, G] SBUF layout (partition-major k-tiles)."""
    K = wT.shape[0]
    return np.ascontiguousarray(
        wT.reshape(K // 128, 128, wT.shape[1]).transpose(1, 0, 2)
    )


def _prep_core_inputs(r, X, weights):
    (w_ih1, w_hh1, b_ih1, b_hh1, w_ih2, w_hh2, b_ih2, b_hh2,
     w_ih3, w_hh3, b_ih3, b_hh3, w_out, b_out) = weights
    f32 = np.float32
    inp = {}
    # X: [B, T, F] -> [T, 128(pad F), B], bf16
    Xt = np.zeros((T, 128, B), f32)
    Xt[:, :F, :] = X.transpose(1, 2, 0)
    inp["Xt"] = Xt.astype(BF16)
    # layer 1 input weight: [512, F].T -> [F, 512] -> pad K to 128
    w1 = _slice_rows(w_ih1, r).T  # [F, 512]
    W1x = np.zeros((128, G), f32)
    W1x[:F] = w1
    inp["W1x"] = W1x.astype(BF16)
    inp["Whh1"] = _k_tiles(_slice_rows(w_hh1, r).T.astype(f32)).astype(BF16)
    inp["Wih2"] = _k_tiles(_slice_rows(w_ih2, r).T.astype(f32)).astype(BF16)
    inp["Whh2"] = _k_tiles(_slice_rows(w_hh2, r).T.astype(f32)).astype(BF16)
    inp["Wih3"] = _k_tiles(_slice_rows(w_ih3, r).T.astype(f32)).astype(BF16)
    inp["Whh3"] = _k_tiles(_slice_rows(w_hh3, r).T.astype(f32)).astype(BF16)

    def brow(bi, bh):
        return _slice_rows((bi + bh).reshape(4 * H, 1), r)[:, 0]  # [512]

    inp["Brows"] = np.stack([
        brow(b_ih1, b_hh1), brow(b_ih2, b_hh2), brow(b_ih3, b_hh3)
    ], 0).astype(BF16).reshape(1, 3, G)  # [1, 3, 512]
    # output projection: w_out [F, H] -> lhsT tiles [128, 8, F]
    inp["Wout"] = _k_tiles(np.ascontiguousarray(w_out.T).astype(f32)).astype(BF16)
    inp["bout"] = b_out.reshape(F, 1).astype(f32)
    return inp


def build_nc(t_steps=T):
    import concourse.bass as bass
    import concourse.mybir as mybir
    import concourse.tile as tile
    from concourse import bacc
    from concourse.masks import make_identity

    f32 = mybir.dt.float32
    bf16 = mybir.dt.bfloat16
    AF = mybir.ActivationFunctionType
    NSLOT = t_steps + 3  # +1 drain slot for the last projection
    rg = [list(range(R))]
    XC = 192  # exchange columns: h1 | h2 | h3 (64 each)

    nc = bacc.Bacc("TRN2", target_bir_lowering=False, debug=False, num_devices=R)

    p_Xt = nc.dram_tensor("Xt", [T, 128, B], bf16, kind="ExternalInput")
    p_W1x = nc.dram_tensor("W1x", [128, G], bf16, kind="ExternalInput")
    pw = {}
    for name in ("Whh1", "Wih2", "Whh2", "Wih3", "Whh3"):
        pw[name] = nc.dram_tensor(name, [128, 8, G], bf16, kind="ExternalInput")
    p_Brows = nc.dram_tensor("Brows", [1, 3, G], bf16, kind="ExternalInput")
    p_Wout = nc.dram_tensor("Wout", [128, 8, F], bf16, kind="ExternalInput")
    p_bout = nc.dram_tensor("bout", [F, 1], f32, kind="ExternalInput")
    p_Y = nc.dram_tensor("Y", [F, t_steps * B], f32, kind="ExternalOutput")

    with tile.TileContext(nc) as tc:
        with (
            tc.tile_pool(name="wpool", bufs=1) as wpool,
            tc.tile_pool(name="state", bufs=1) as state,
            tc.tile_pool(name="xq", bufs=4) as xq,
            tc.tile_pool(name="gq", bufs=2) as gq,
            tc.tile_pool(name="yst", bufs=2) as yst,
            tc.tile_pool(name="snd", bufs=3) as snd,
            tc.tile_pool(name="sbt", bufs=2) as sbt,
            tc.tile_pool(name="gps", bufs=3, space="PSUM") as gps,
            tc.tile_pool(name="tps", bufs=2, space="PSUM") as tps,
            tc.tile_pool(name="yps", bufs=2, space="PSUM") as yps,
            tc.tile_pool(name="dms", bufs=3, space="DRAM") as dms,
        ):
            # ---- resident weights (bf16) ----
            W1x = wpool.tile([128, G], bf16, tag="W1x")
            nc.sync.dma_start(W1x[:], p_W1x[:])
            W = {}
            for name in ("Whh1", "Wih2", "Whh2", "Wih3", "Whh3"):
                W[name] = wpool.tile([128, 8, G], bf16, tag=name, name=name + "_sb")
                nc.sync.dma_start(W[name][:], pw[name][:])
            Brows = wpool.tile([1, 3, G], bf16, tag="Brows", name="Brows_sb")
            nc.sync.dma_start(Brows[:], p_Brows[:])
            ones1 = wpool.tile([1, 2 * B], bf16, tag="ones1")
            nc.gpsimd.memset(ones1[:], 1.0)
            e2 = wpool.tile([1, 2 * B], bf16, tag="e2")
            nc.gpsimd.memset(e2[:, 0:B], 1.0)
            nc.gpsimd.memset(e2[:, B:2 * B], 0.0)
            e3 = wpool.tile([1, 2 * B], bf16, tag="e3")
            nc.gpsimd.memset(e3[:, 0:B], 0.0)
            nc.gpsimd.memset(e3[:, B:2 * B], 1.0)
            Wout = wpool.tile([128, 8, F], bf16, tag="Wout")
            nc.sync.dma_start(Wout[:], p_Wout[:])
            bout = wpool.tile([F, 1], f32, tag="bout")
            nc.sync.dma_start(bout[:], p_bout[:])
            ident = wpool.tile([128, 128], bf16, tag="ident")
            make_identity(nc, ident[:])

            # ---- persistent state ----
            c1t = state.tile([B, HS], f32, tag="c1", name="c1")
            nc.gpsimd.memset(c1t[:], 0.0)
            c23t = state.tile([2 * B, HS], f32, tag="c23", name="c23")
            nc.gpsimd.memset(c23t[:], 0.0)
            cst = {1: c1t, 2: c23t[0:B], 3: c23t[B:2 * B]}

            # gathered exchange tiles: Gt[s] = [128, 8, 192] bf16, cols
            # 0:64 h1(s), 64:128 h2(s-1), 128:192 h3(s-2)
            Gt = {}
            ystage = {}  # group idx -> [F, YG*B] f32 staging for output

            def lstm_elementwise(key, gpsum, c, P):
                """gates psum [P, G] (bias pre-accumulated) -> h bf16 [P, HS]."""
                sio = sbt.tile([P, 3 * HS], f32, tag=f"sio{key}", name=f"sio{key}")
                nc.scalar.activation(sio[:], gpsum[:, 0:3 * HS], AF.Sigmoid)
                tg = sbt.tile([P, HS], f32, tag=f"tg{key}", name=f"tg{key}")
                nc.scalar.activation(tg[:], gpsum[:, 3 * HS:4 * HS], AF.Tanh)
                fc = sbt.tile([P, HS], f32, tag=f"fc{key}", name=f"fc{key}")
                nc.vector.tensor_mul(out=fc[:], in0=sio[:, HS:2 * HS], in1=c)
                ig = sbt.tile([P, HS], f32, tag=f"ig{key}", name=f"ig{key}")
                nc.vector.tensor_mul(out=ig[:], in0=sio[:, 0:HS], in1=tg[:])
                nc.vector.tensor_add(out=c, in0=fc[:], in1=ig[:])
                tc_ = sbt.tile([P, HS], f32, tag=f"tc{key}", name=f"tc{key}")
                nc.scalar.activation(tc_[:], c, AF.Tanh)
                h = sbt.tile([P, HS], bf16, tag=f"h{key}", name=f"h{key}")
                nc.vector.tensor_mul(out=h[:], in0=sio[:, 2 * HS:3 * HS], in1=tc_[:])
                return h

            for s in range(NSLOT):
                t2, t3 = s - 1, s - 2
                l1_active = s < t_steps
                l2_active = 0 <= t2 < t_steps
                l3_active = 0 <= t3 < t_steps
                packed = l2_active and l3_active
                Gp = Gt.get(s - 1)  # gathered exchange from previous slot

                # ---------- layer 1 matmuls: t = s ----------
                if l1_active:
                    xs = xq.tile([128, B], bf16)
                    nc.sync.dma_start(xs[:], p_Xt[s])
                    g1 = gps.tile([2 * B, G], f32, tag="g", name="g1")[0:B]
                    nc.tensor.matmul(g1[:], ones1[:, 0:B], Brows[:, 0], start=True,
                                     stop=False)
                    nc.tensor.matmul(g1[:], xs[:], W1x[:], start=False, stop=(s == 0))
                    if s > 0:
                        for k in range(8):
                            nc.tensor.matmul(g1[:], Gp[:, k, 0:64], W["Whh1"][:, k],
                                             start=False, stop=(k == 7))

                # ---------- layers 2+3 matmuls (packed when both active) ----
                if packed:
                    g23 = gps.tile([2 * B, G], f32, tag="g", name="g23")
                    gl2 = g23[0:B]
                    gl3 = g23[B:2 * B]
                    nc.tensor.matmul(g23[:], e2[:], Brows[:, 1], start=True,
                                     stop=False, skip_group_check=True)
                    nc.tensor.matmul(g23[:], e3[:], Brows[:, 2], start=False,
                                     stop=False, skip_group_check=True)
                    # interleave base-0 (L2) and base-64 (L3) matmuls so they
                    # run in different PE column groups concurrently
                    l2_mms = [(Gp[:, k, 0:64], W["Wih2"][:, k]) for k in range(8)]
                    if t2 > 0:
                        l2_mms += [(Gp[:, k, 64:128], W["Whh2"][:, k]) for k in range(8)]
                    l3_mms = [(Gp[:, k, 64:128], W["Wih3"][:, k]) for k in range(8)]
                    if t3 > 0:
                        l3_mms += [(Gp[:, k, 128:192], W["Whh3"][:, k]) for k in range(8)]
                    n = max(len(l2_mms), len(l3_mms))
                    for i in range(n):
                        if i < len(l2_mms):
                            lhsT, rhs = l2_mms[i]
                            nc.tensor.matmul(gl2, lhsT, rhs, start=False,
                                             stop=(i == len(l2_mms) - 1),
                                             skip_group_check=True)
                        if i < len(l3_mms):
                            lhsT, rhs = l3_mms[i]
                            nc.tensor.matmul(gl3, lhsT, rhs, start=False,
                                             stop=(i == len(l3_mms) - 1),
                                             skip_group_check=True)
                elif l2_active:  # s == 1 (first L2 step) or tail
                    g2 = gps.tile([2 * B, G], f32, tag="g", name="g2")[0:B]
                    nc.tensor.matmul(g2[:], ones1[:, 0:B], Brows[:, 1], start=True,
                                     stop=False)
                    for k in range(8):
                        nc.tensor.matmul(g2[:], Gp[:, k, 0:64], W["Wih2"][:, k],
                                         start=False, stop=(t2 == 0 and k == 7))
                    if t2 > 0:
                        for k in range(8):
                            nc.tensor.matmul(g2[:], Gp[:, k, 64:128], W["Whh2"][:, k],
                                             start=False, stop=(k == 7))
                elif l3_active:  # s == T+1 (last L3 step)
                    g3 = gps.tile([2 * B, G], f32, tag="g", name="g3")[0:B]
                    nc.tensor.matmul(g3[:], ones1[:, 0:B], Brows[:, 2], start=True,
                                     stop=False)
                    for k in range(8):
                        nc.tensor.matmul(g3[:], Gp[:, k, 64:128], W["Wih3"][:, k],
                                         start=False, stop=False)
                    for k in range(8):
                        nc.tensor.matmul(g3[:], Gp[:, k, 128:192], W["Whh3"][:, k],
                                         start=False, stop=(k == 7))

                # ---------- elementwise ----------
                sendbuf = None
                if s <= t_steps + 1:
                    sendbuf = snd.tile([128, XC], bf16, tag="snd", name="sendbuf")
                wrote_lo = wrote_hi = False
                pt = tps.tile([HS, XC], bf16, tag="pt", name="pt") \
                    if sendbuf is not None else None
                if l1_active:
                    h1 = lstm_elementwise("1", g1, cst[1], B)
                    nc.tensor.transpose(pt[:, 0:64], h1[:], ident[0:B, 0:B])
                    wrote_lo = True
                if packed:
                    h23 = lstm_elementwise("23", g23, c23t[:], 2 * B)
                    nc.tensor.transpose(pt[:, 64:192], h23[:], ident[:])
                    wrote_hi = True
                elif l2_active:
                    h2 = lstm_elementwise("2", g2, cst[2], B)
                    nc.tensor.transpose(pt[:, 64:128], h2[:], ident[0:B, 0:B])
                elif l3_active:
                    # last L3 step: c3 lives at partitions 64-127; copy down
                    c3tmp = sbt.tile([B, HS], f32, tag="c3tmp", name="c3tmp")
                    nc.sync.dma_start(c3tmp[:], cst[3])
                    h3 = lstm_elementwise("3", g3, c3tmp[:], B)
                    nc.tensor.transpose(pt[:, 128:192], h3[:], ident[0:B, 0:B])
                if sendbuf is not None and (l1_active or l2_active or l3_active):
                    nc.vector.tensor_copy(out=sendbuf[:], in_=pt[:])
                if sendbuf is not None:
                    if not wrote_lo:
                        nc.gpsimd.memset(sendbuf[:, 0:64], 0.0)
                    if not wrote_hi and not (l2_active and not packed):
                        nc.gpsimd.memset(sendbuf[:, 64:192] if not l3_active or packed
                                         else sendbuf[:, 64:128], 0.0)
                    elif l2_active and not packed:
                        nc.gpsimd.memset(sendbuf[:, 128:192], 0.0)

                # ---------- single merged AllGather ----------
                if s <= t_steps + 1:
                    agin = dms.tile([128, XC], bf16, tag="agin", name="agin")
                    nc.sync.dma_start(agin[:], sendbuf[:])
                    agout = dms.tile([R, 128, XC], bf16, tag="agout", name="agout")
                    nc.gpsimd.collective_compute(
                        "AllGather", mybir.AluOpType.bypass,
                        replica_groups=rg, ins=[agin[:].opt()], outs=[agout[:].opt()],
                    )
                    Gn = gq.tile([128, 8, XC], bf16, tag="G", name="Gt")
                    nc.scalar.dma_start(Gn[:], agout[:].rearrange("r p c -> p r c"))
                    Gt[s] = Gn
                    Gt.pop(s - 2, None)

                # ---------- output projection (t = s-3, from Gp h3 cols) ----
                tp = s - 3
                if 0 <= tp < t_steps:
                    g_, j_ = tp // YG, tp % YG
                    if j_ == 0:
                        ystage[g_] = yst.tile([F, YG * B], f32, tag="yst",
                                              name="ystage")
                    yp = yps.tile([F, B], f32)
                    for k in range(8):
                        nc.tensor.matmul(yp[:], Wout[:, k], Gp[:, k, 128:192],
                                         start=(k == 0), stop=(k == 7))
                    nc.scalar.activation(ystage[g_][:, j_ * B:(j_ + 1) * B],
                                         yp[:], AF.Tanh, bias=bout[:])
                    if j_ == YG - 1:
                        nc.sync.dma_start(
                            p_Y[:, g_ * YG * B:(g_ + 1) * YG * B], ystage[g_][:])
                        ystage.pop(g_, None)

    nc.compile()
    return nc


_CACHED = {}


def _get_nc(t_steps=T):
    if t_steps not in _CACHED:
        _CACHED[t_steps] = build_nc(t_steps)
    return _CACHED[t_steps]


def make_in_maps(X, weights):
    return [_prep_core_inputs(r, X, weights) for r in range(R)]


def _weights_tuple(kw):
    return tuple(
        np.asarray(kw[k], np.float32)
        for k in ("w_ih1", "w_hh1", "b_ih1", "b_hh1", "w_ih2", "w_hh2", "b_ih2",
                  "b_hh2", "w_ih3", "w_hh3", "b_ih3", "b_hh3", "w_out", "b_out")
    )


def assemble_output(Y, t_steps=T):
    """[F, t*B] -> [B, t, F]"""
    return np.ascontiguousarray(Y.reshape(F, t_steps, B).transpose(2, 1, 0))


def kernel(X, **kw):
    from concourse.bass_utils import run_bass_kernel_spmd

    nc = _get_nc(T)
    in_maps = make_in_maps(np.asarray(X, np.float32), _weights_tuple(kw))
    res = run_bass_kernel_spmd(nc, in_maps, core_ids=list(range(R)))
    return assemble_output(res.results[0]["Y"])


# revision 11
# speedup vs baseline: 1.3381x; 1.3381x over previous
"""Trainium2 Bass kernel for a 3-layer LSTM (B=64, T=256, F=64, H=1024)
+ tanh output projection, SPMD across 8 NeuronCores.

Strategy: shard the 4H=4096 gate dimension 8 ways (each core owns a
512-wide gate slice == a 128-wide h-slice per layer), keep the full batch
B=64 on every core. Per time step each core computes its gate slice
(h.T stationary [128,64] x weight moving [128,512] matmuls, PSUM
accumulated over K), does the LSTM elementwise on its slice, transposes
its h-slice to [128,B] via the PE, and exchanges it so every core has
the full h.T for the next step. The 3 layers are software-pipelined:
slot s computes L1(t=s), L2(t=s-1), L3(t=s-2). All three layers'
transposed h-slices for a slot are packed into ONE [128,192] buffer and
exchanged with a single AllGather per slot (collective dispatch
overhead dominates, so 1 merged AG/slot beats 3 small ones 3x). All
matmul operands are bf16 (fp32 matmul runs at 1/4 rate on the PE);
accumulation, elementwise and state stay fp32. The output projection
runs every 8 slots from SBUF-resident gathered h3.
"""

import sys

sys.path.insert(0, "/opt/trn_rl_repo")

import numpy as np
import ml_dtypes

BF16 = ml_dtypes.bfloat16

B, T, F, H = 64, 256, 64, 1024
R = 8           # cores
G = 4 * H // R  # 512 gate slice per core
HS = H // R     # 128 h slice per core
YG = 8          # y-projection group size (slots)

_GATE_ORDER = (0, 1, 3, 2)  # i, f, o, g (PyTorch row blocks i,f,g,o)


def _slice_rows(w, r):
    """Rows of a [4H, *] weight for core r, in i|f|o|g block order."""
    return np.concatenate([w[g * H + HS * r: g * H + HS * (r + 1)] for g in _GATE_ORDER], 0)


def _k_tiles(wT):
    """[K, G] -> [128, K/128, G] SBUF layout (partition-major k-tiles)."""
    K = wT.shape[0]
    return np.ascontiguousarray(
        wT.reshape(K // 128, 128, wT.shape[1]).transpose(1, 0, 2)
    )


def _prep_core_inputs(r, X, weights):
    (w_ih1, w_hh1, b_ih1, b_hh1, w_ih2, w_hh2, b_ih2, b_hh2,
     w_ih3, w_hh3, b_ih3, b_hh3, w_out, b_out) = weights
    f32 = np.float32
    inp = {}
    # X: [B, T, F] -> [T, 128(pad F), B], bf16
    Xt = np.zeros((T, 128, B), f32)
    Xt[:, :F, :] = X.transpose(1, 2, 0)
    inp["Xt"] = Xt.astype(BF16)
    # layer 1 input weight: [512, F].T -> [F, 512] -> pad K to 128
    w1 = _slice_rows(w_ih1, r).T  # [F, 512]
    W1x = np.zeros((128, G), f32)
    W1x[:F] = w1
    inp["W1x"] = W1x.astype(BF16)
    inp["Whh1"] = _k_tiles(_slice_rows(w_hh1, r).T.astype(f32)).astype(BF16)
    inp["Wih2"] = _k_tiles(_slice_rows(w_ih2, r).T.astype(f32)).astype(BF16)
    inp["Whh2"] = _k_tiles(_slice_rows(w_hh2, r).T.astype(f32)).astype(BF16)
    inp["Wih3"] = _k_tiles(_slice_rows(w_ih3, r).T.astype(f32)).astype(BF16)
    inp["Whh3"] = _k_tiles(_slice_rows(w_hh3, r).T.astype(f32)).astype(BF16)

    def brow(bi, bh):
        return _slice_rows((bi + bh).reshape(4 * H, 1), r)[:, 0]  # [512]

    inp["Brows"] = np.stack([
        brow(b_ih1, b_hh1), brow(b_ih2, b_hh2), brow(b_ih3, b_hh3)
    ], 0).astype(BF16).reshape(1, 3, G)  # [1, 3, 512]
    # output projection: w_out [F, H] -> lhsT tiles [128, 8, F]
    inp["Wout"] = _k_tiles(np.ascontiguousarray(w_out.T).astype(f32)).astype(BF16)
    inp["bout"] = b_out.reshape(F, 1).astype(f32)
    return inp


def build_nc(t_steps=T):
    import concourse.bass as bass
    import concourse.mybir as mybir
    import concourse.tile as tile
    from concourse import bacc
    from concourse.masks import make_identity

    f32 = mybir.dt.float32
    bf16 = mybir.dt.bfloat16
    AF = mybir.ActivationFunctionType
    NSLOT = t_steps + 3  # +1 drain slot for the last projection
    rg = [list(range(R))]
    XC = 192  # exchange columns: h1 | h2 | h3 (64 each)

    nc = bacc.Bacc("TRN2", target_bir_lowering=False, debug=False, num_devices=R)

    p_Xt = nc.dram_tensor("Xt", [T, 128, B], bf16, kind="ExternalInput")
    p_W1x = nc.dram_tensor("W1x", [128, G], bf16, kind="ExternalInput")
    pw = {}
    for name in ("Whh1", "Wih2", "Whh2", "Wih3", "Whh3"):
        pw[name] = nc.dram_tensor(name, [128, 8, G], bf16, kind="ExternalInput")
    p_Brows = nc.dram_tensor("Brows", [1, 3, G], bf16, kind="ExternalInput")
    p_Wout = nc.dram_tensor("Wout", [128, 8, F], bf16, kind="ExternalInput")
    p_bout = nc.dram_tensor("bout", [F, 1], f32, kind="ExternalInput")
    p_Y = nc.dram_tensor("Y", [F, t_steps * B], f32, kind="ExternalOutput")

    with tile.TileContext(nc) as tc:
        with (
            tc.tile_pool(name="wpool", bufs=1) as wpool,
            tc.tile_pool(name="state", bufs=1) as state,
            tc.tile_pool(name="xq", bufs=4) as xq,
            tc.tile_pool(name="gq", bufs=2) as gq,
            tc.tile_pool(name="yst", bufs=2) as yst,
            tc.tile_pool(name="snd", bufs=3) as snd,
            tc.tile_pool(name="sbt", bufs=2) as sbt,
            tc.tile_pool(name="gps", bufs=3, space="PSUM") as gps,
            tc.tile_pool(name="tps", bufs=2, space="PSUM") as tps,
            tc.tile_pool(name="yps", bufs=2, space="PSUM") as yps,
            tc.tile_pool(name="dms", bufs=3, space="DRAM") as dms,
        ):
            # ---- resident weights (bf16) ----
            W1x = wpool.tile([128, G], bf16, tag="W1x")
            nc.sync.dma_start(W1x[:], p_W1x[:])
            W = {}
            for name in ("Whh1", "Wih2", "Whh2", "Wih3", "Whh3"):
                W[name] = wpool.tile([128, 8, G], bf16, tag=name, name=name + "_sb")
                nc.sync.dma_start(W[name][:], pw[name][:])
            Brows = wpool.tile([1, 3, G], bf16, tag="Brows", name="Brows_sb")
            nc.sync.dma_start(Brows[:], p_Brows[:])
            ones1 = wpool.tile([1, 2 * B], bf16, tag="ones1")
            nc.gpsimd.memset(ones1[:], 1.0)
            e2 = wpool.tile([1, 2 * B], bf16, tag="e2")
            nc.gpsimd.memset(e2[:, 0:B], 1.0)
            nc.gpsimd.memset(e2[:, B:2 * B], 0.0)
            e3 = wpool.tile([1, 2 * B], bf16, tag="e3")
            nc.gpsimd.memset(e3[:, 0:B], 0.0)
            nc.gpsimd.memset(e3[:, B:2 * B], 1.0)
            Wout = wpool.tile([128, 8, F], bf16, tag="Wout")
            nc.sync.dma_start(Wout[:], p_Wout[:])
            bout = wpool.tile([F, 1], f32, tag="bout")
            nc.sync.dma_start(bout[:], p_bout[:])
            ident = wpool.tile([128, 128], bf16, tag="ident")
            make_identity(nc, ident[:])

            # ---- persistent state ----
            c1t = state.tile([B, HS], f32, tag="c1", name="c1")
            nc.gpsimd.memset(c1t[:], 0.0)
            c23t = state.tile([2 * B, HS], f32, tag="c23", name="c23")
            nc.gpsimd.memset(c23t[:], 0.0)
            cst = {1: c1t, 2: c23t[0:B], 3: c23t[B:2 * B]}

            # gathered exchange tiles: Gt[s] = [128, 8, 192] bf16, cols
            # 0:64 h1(s), 64:128 h2(s-1), 128:192 h3(s-2)
            Gt = {}
            ystage = {}  # group idx -> [F, YG*B] f32 staging for output

            def lstm_elementwise(key, gpsum, c, P):
                """gates psum [P, G] (bias pre-accumulated) -> h bf16 [P, HS]."""
                sio = sbt.tile([P, 3 * HS], f32, tag=f"sio{key}", name=f"sio{key}")
                nc.scalar.activation(sio[:], gpsum[:, 0:3 * HS], AF.Sigmoid)
                tg = sbt.tile([P, HS], f32, tag=f"tg{key}", name=f"tg{key}")
                nc.scalar.activation(tg[:], gpsum[:, 3 * HS:4 * HS], AF.Tanh)
                fc = sbt.tile([P, HS], f32, tag=f"fc{key}", name=f"fc{key}")
                nc.vector.tensor_mul(out=fc[:], in0=sio[:, HS:2 * HS], in1=c)
                ig = sbt.tile([P, HS], f32, tag=f"ig{key}", name=f"ig{key}")
                nc.vector.tensor_mul(out=ig[:], in0=sio[:, 0:HS], in1=tg[:])
                nc.vector.tensor_add(out=c, in0=fc[:], in1=ig[:])
                tc_ = sbt.tile([P, HS], f32, tag=f"tc{key}", name=f"tc{key}")
                nc.scalar.activation(tc_[:], c, AF.Tanh)
                h = sbt.tile([P, HS], bf16, tag=f"h{key}", name=f"h{key}")
                nc.vector.tensor_mul(out=h[:], in0=sio[:, 2 * HS:3 * HS], in1=tc_[:])
                return h

            for s in range(NSLOT):
                t2, t3 = s - 1, s - 2
                l1_active = s < t_steps
                l2_active = 0 <= t2 < t_steps
                l3_active = 0 <= t3 < t_steps
                packed = l2_active and l3_active
                Gp = Gt.get(s - 1)  # gathered exchange from previous slot

                # ---------- layer 1 matmuls: t = s ----------
                if l1_active:
                    xs = xq.tile([128, B], bf16)
                    nc.sync.dma_start(xs[:], p_Xt[s])
                    g1 = gps.tile([2 * B, G], f32, tag="g", name="g1")[0:B]
                    nc.tensor.matmul(g1[:], ones1[:, 0:B], Brows[:, 0], start=True,
                                     stop=False)
                    nc.tensor.matmul(g1[:], xs[:], W1x[:], start=False, stop=(s == 0))
                    if s > 0:
                        for k in range(8):
                            nc.tensor.matmul(g1[:], Gp[:, k, 0:64], W["Whh1"][:, k],
                                             start=False, stop=(k == 7))

                # ---------- layers 2+3 matmuls (packed when both active) ----
                if packed:
                    g23 = gps.tile([2 * B, G], f32, tag="g", name="g23")
                    gl2 = g23[0:B]
                    gl3 = g23[B:2 * B]
                    nc.tensor.matmul(g23[:], e2[:], Brows[:, 1], start=True,
                                     stop=False, skip_group_check=True)
                    nc.tensor.matmul(g23[:], e3[:], Brows[:, 2], start=False,
                                     stop=False, skip_group_check=True)
                    # interleave base-0 (L2) and base-64 (L3) matmuls so they
                    # run in different PE column groups concurrently
                    l2_mms = [(Gp[:, k, 0:64], W["Wih2"][:, k]) for k in range(8)]
                    if t2 > 0:
                        l2_mms += [(Gp[:, k, 64:128], W["Whh2"][:, k]) for k in range(8)]
                    l3_mms = [(Gp[:, k, 64:128], W["Wih3"][:, k]) for k in range(8)]
                    if t3 > 0:
                        l3_mms += [(Gp[:, k, 128:192], W["Whh3"][:, k]) for k in range(8)]
                    n = max(len(l2_mms), len(l3_mms))
                    for i in range(n):
                        if i < len(l2_mms):
                            lhsT, rhs = l2_mms[i]
                            nc.tensor.matmul(gl2, lhsT, rhs, start=False,
                                             stop=(i == len(l2_mms) - 1),
                                             skip_group_check=True)
                        if i < len(l3_mms):
                            lhsT, rhs = l3_mms[i]
                            nc.tensor.matmul(gl3, lhsT, rhs, start=False,
                                             stop=(i == len(l3_mms) - 1),
                                             skip_group_check=True)
                elif l2_active:  # s == 1 (first L2 step) or tail
                    g2 = gps.tile([2 * B, G], f32, tag="g", name="g2")[0:B]
                    nc.tensor.matmul(g2[:], ones1[:, 0:B], Brows[:, 1], start=True,
                                     stop=False)
                    for k in range(8):
                        nc.tensor.matmul(g2[:], Gp[:, k, 0:64], W["Wih2"][:, k],
                                         start=False, stop=(t2 == 0 and k == 7))
                    if t2 > 0:
                        for k in range(8):
                            nc.tensor.matmul(g2[:], Gp[:, k, 64:128], W["Whh2"][:, k],
                                             start=False, stop=(k == 7))
                elif l3_active:  # s == T+1 (last L3 step)
                    g3 = gps.tile([2 * B, G], f32, tag="g", name="g3")[0:B]
                    nc.tensor.matmul(g3[:], ones1[:, 0:B], Brows[:, 2], start=True,
                                     stop=False)
                    for k in range(8):
                        nc.tensor.matmul(g3[:], Gp[:, k, 64:128], W["Wih3"][:, k],
                                         start=False, stop=False)
                    for k in range(8):
                        nc.tensor.matmul(g3[:], Gp[:, k, 128:192], W["Whh3"][:, k],
                                         start=False, stop=(k == 7))

                # ---------- elementwise ----------
                sendbuf = None
                if s <= t_steps + 1:
                    sendbuf = snd.tile([128, XC], bf16, tag="snd", name="sendbuf")
                wrote_lo = wrote_hi = False
                pt = tps.tile([HS, XC], bf16, tag="pt", name="pt") \
                    if sendbuf is not None else None
                if l1_active:
                    h1 = lstm_elementwise("1", g1, cst[1], B)
                    nc.tensor.transpose(pt[:, 0:64], h1[:], ident[0:B, 0:B])
                    wrote_lo = True
                if packed:
                    h23 = lstm_elementwise("23", g23, c23t[:], 2 * B)
                    nc.tensor.transpose(pt[:, 64:192], h23[:], ident[:])
                    wrote_hi = True
                elif l2_active:
                    h2 = lstm_elementwise("2", g2, cst[2], B)
                    nc.tensor.transpose(pt[:, 64:128], h2[:], ident[0:B, 0:B])
                elif l3_active:
                    # last L3 step: c3 lives at partitions 64-127; copy down
                    c3tmp = sbt.tile([B, HS], f32, tag="c3tmp", name="c3tmp")
                    nc.sync.dma_start(c3tmp[:], cst[3])
                    h3 = lstm_elementwise("3", g3, c3tmp[:], B)
                    nc.tensor.transpose(pt[:, 128:192], h3[:], ident[0:B, 0:B])
                if sendbuf is not None and (l1_active or l2_active or l3_active):
                    nc.vector.tensor_copy(out=sendbuf[:], in_=pt[:])
                if sendbuf is not None:
                    if not wrote_lo:
                        nc.gpsimd.memset(sendbuf[:, 0:64], 0.0)
                    if not wrote_hi and not (l2_active and not packed):
                        nc.gpsimd.memset(sendbuf[:, 64:192] if not l3_active or packed
                                         else sendbuf[:, 64:128], 0.0)
                    elif l2_active and not packed:
                        nc.gpsimd.memset(sendbuf[:, 128:192], 0.0)

                # ---------- single merged AllGather ----------
                if s <= t_steps + 1:
                    agin = dms.tile([128, XC], bf16, tag="agin", name="agin")
                    nc.sync.dma_start(agin[:], sendbuf[:])
                    agout = dms.tile([R, 128, XC], bf16, tag="agout", name="agout")
                    nc.gpsimd.collective_compute(
                        "AllGather", mybir.AluOpType.bypass,
                        replica_groups=rg, ins=[agin[:].opt()], outs=[agout[:].opt()],
                    )
                    Gn = gq.tile([128, 8, XC], bf16, tag="G", name="Gt")
                    nc.gpsimd.dma_start(Gn[:], agout[:].rearrange("r p c -> p r c"))
                    Gt[s] = Gn
                    Gt.pop(s - 2, None)

                # ---------- output projection (t = s-3, from Gp h3 cols) ----
                tp = s - 3
                if 0 <= tp < t_steps:
                    g_, j_ = tp // YG, tp % YG
                    if j_ == 0:
                        ystage[g_] = yst.tile([F, YG * B], f32, tag="yst",
                                              name="ystage")
                    yp = yps.tile([F, B], f32)
                    for k in range(8):
                        nc.tensor.matmul(yp[:], Wout[:, k], Gp[:, k, 128:192],
                                         start=(k == 0), stop=(k == 7))
                    nc.scalar.activation(ystage[g_][:, j_ * B:(j_ + 1) * B],
                                         yp[:], AF.Tanh, bias=bout[:])
                    if j_ == YG - 1:
                        nc.sync.dma_start(
                            p_Y[:, g_ * YG * B:(g_ + 1) * YG * B], ystage[g_][:])
                        ystage.pop(g_, None)

    nc.compile()
    return nc


_CACHED = {}


def _get_nc(t_steps=T):
    if t_steps not in _CACHED:
        _CACHED[t_steps] = build_nc(t_steps)
    return _CACHED[t_steps]


def make_in_maps(X, weights):
    return [_prep_core_inputs(r, X, weights) for r in range(R)]


def _weights_tuple(kw):
    return tuple(
        np.asarray(kw[k], np.float32)
        for k in ("w_ih1", "w_hh1", "b_ih1", "b_hh1", "w_ih2", "w_hh2", "b_ih2",
                  "b_hh2", "w_ih3", "w_hh3", "b_ih3", "b_hh3", "w_out", "b_out")
    )


def assemble_output(Y, t_steps=T):
    """[F, t*B] -> [B, t, F]"""
    return np.ascontiguousarray(Y.reshape(F, t_steps, B).transpose(2, 1, 0))


def kernel(X, **kw):
    from concourse.bass_utils import run_bass_kernel_spmd

    nc = _get_nc(T)
    in_maps = make_in_maps(np.asarray(X, np.float32), _weights_tuple(kw))
    res = run_bass_kernel_spmd(nc, in_maps, core_ids=list(range(R)))
    return assemble_output(res.results[0]["Y"])
